# revision 4
# baseline (speedup 1.0000x reference)
"""2-layer GCN forward (PyG GCNConv semantics) on 8 Trainium2 NeuronCores.

Strategy (graph/data parallel, per sharding hint):
  - Nodes (rows of x / output) sharded by dst across 8 cores (12544-row shards).
  - Edges sharded by destination core; within a core, grouped by
    (dst-tile of 128 nodes, src-block of 25000 nodes) and padded to
    128-edge chunks.
  - Aggregation (segment_sum of norm-scaled neighbor rows) is computed as a
    sequence of TensorE matmuls: for each 128-edge chunk, gather the source
    rows with dma_gather (SWDGE indirect gather), build a sparse scatter
    matrix S[e, d] = (iota[d] == dst_local[e]) * norm[e] with one
    tensor_scalar op, and accumulate Xg.T @ S into PSUM.
  - Layer 1 aggregates x first (A@x)@W1 (128-wide gathers instead of
    512-wide), layer 2 projects first (A@(h@W2)) like the reference.
  - hidden = relu(agg1@W1 + b1) * dropout_mask is written out per shard;
    p = hidden@W2 is AllGathered so layer-2 gathers are local.
"""

import functools
import os

import numpy as np

import concourse.bass as bass
import concourse.mybir as mybir
import concourse.bacc as bacc
import concourse.tile as tile
from concourse import bass_utils
from concourse.bass_interp import get_hw_module

F32 = mybir.dt.float32
I16 = mybir.dt.int16
AL = mybir.AluOpType

NCORES = 8
P = 128


class Cfg:
    def __init__(self, n, e, hid=512, o=40, g=2048):
        self.N = n
        self.E = e
        self.F = 128
        self.HID = hid
        self.O = o
        self.OPAD = 64
        # per-core dst shard, padded to a multiple of P
        self.SHARD = ((n + NCORES - 1) // NCORES + P - 1) // P * P
        self.T = self.SHARD // P  # dst tiles per core
        self.NBLK = 4
        self.BLK = (n + self.NBLK - 1) // self.NBLK  # src block size (int16-safe)
        assert self.BLK <= 32767
        self.G = g  # idxs per dma_gather call
        self.CPB = g // P  # chunks per gather call


CFG = Cfg(100000, 800000)


# ----------------------------------------------------------------------------
# Host-side preprocessing: normalization, sharding, chunk schedule, streams
# ----------------------------------------------------------------------------

def preprocess(cfg, edge_index, x, W1, b1, W2, b2, mask2):
    N, E = cfg.N, cfg.E
    src0 = edge_index[0].astype(np.int64)
    dst0 = edge_index[1].astype(np.int64)
    loops = np.arange(N, dtype=np.int64)
    src = np.concatenate([src0, loops])
    dst = np.concatenate([dst0, loops])
    deg = np.bincount(dst, minlength=N).astype(np.float32)
    dis = (1.0 / np.sqrt(deg)).astype(np.float32)
    norm = (dis[src] * dis[dst]).astype(np.float32)

    core = dst // cfg.SHARD
    tloc = (dst - core * cfg.SHARD) // P
    dloc = dst - core * cfg.SHARD - tloc * P  # 0..127 within tile
    blk = src // cfg.BLK

    T, NBLK = cfg.T, cfg.NBLK
    key = (core * T + tloc) * NBLK + blk
    counts = np.bincount(key, minlength=NCORES * T * NBLK).reshape(NCORES, T, NBLK)
    C = -(-counts // P)  # ceil
    C = C.max(axis=0)  # [T, NBLK] uniform chunk counts across cores
    # every tile must have at least one chunk so PSUM gets written
    for t in range(T):
        if C[t].sum() == 0:
            C[t][0] = 1

    nch = int(C.sum())
    # stream-chunk start of (t, b) within block b's stream
    start = np.zeros((T, NBLK), np.int64)
    for b in range(NBLK):
        start[1:, b] = np.cumsum(C[:-1, b])
    chunks_per_block = C.sum(axis=0)  # [NBLK]
    Lb = chunks_per_block * P
    Lb_pad = -(-Lb // cfg.G) * cfg.G
    calls_b = Lb_pad // cfg.G
    off16 = np.zeros(NBLK + 1, np.int64)
    off16[1:] = np.cumsum(Lb_pad // 16)
    idx_cols = int(off16[-1])

    # global chunk id in schedule order (t-major, then b, then k)
    gid_start = np.zeros((T, NBLK), np.int64)
    run = 0
    for t in range(T):
        for b in range(NBLK):
            gid_start[t, b] = run
            run += C[t, b]
    assert run == nch

    # schedule for the program builder: per tile, list of (b, gid, block_chunk)
    schedule = []
    for t in range(T):
        ch = []
        for b in range(NBLK):
            for k in range(int(C[t, b])):
                ch.append((b, int(gid_start[t, b] + k), int(start[t, b] + k)))
        schedule.append(ch)

    # per-core data streams
    order = np.argsort(key, kind="stable")
    idx16 = np.zeros((NCORES, 16, idx_cols), np.int16)
    dn = np.zeros((NCORES, P, 2 * nch), np.float32)
    dn[:, :, 0::2] = 200.0  # padding dst_local: never matches iota 0..127

    ksort = key[order]
    group_first = np.searchsorted(ksort, np.arange(NCORES * T * NBLK), side="left")
    rank = np.arange(len(order)) - group_first[ksort]  # rank within (c,t,b) group

    e_core = core[order]
    e_t = tloc[order]
    e_b = blk[order]
    e_src = src[order]
    e_dloc = dloc[order].astype(np.float32)
    e_norm = norm[order]

    # stream slot within block stream, chunk ids
    slot = start[e_t, e_b] * P + rank
    gidv = gid_start[e_t, e_b] + rank // P
    lane = (rank % P).astype(np.int64)
    col16 = off16[e_b] + slot // 16
    row16 = slot % 16
    idxval = (e_src - e_b * cfg.BLK).astype(np.int16)

    for c in range(NCORES):
        m = e_core == c
        idx16[c, row16[m], col16[m]] = idxval[m]
        dn[c, lane[m], 2 * gidv[m]] = e_dloc[m]
        dn[c, lane[m], 2 * gidv[m] + 1] = e_norm[m]

    idx128 = np.tile(idx16, (1, 8, 1))  # replicate 16-row pattern across 128 parts

    # constants
    iota = np.tile(np.arange(P, dtype=np.float32)[None, :], (P, 1))
    ident = np.eye(P, dtype=np.float32)
    KH = cfg.HID // P
    W2p = np.zeros((cfg.HID, cfg.OPAD), np.float32)
    W2p[:, : cfg.O] = W2
    W2p = W2p.reshape(KH, P, cfg.OPAD).transpose(1, 0, 2).copy()  # [P, KH, OPAD]

    b1_nonzero = bool(np.any(b1 != 0))
    b1b = np.tile(b1.astype(np.float32)[None, :], (P, 1))

    # per-core masks {0,2}, f32, padded to SHARD rows
    masks = np.zeros((NCORES, cfg.SHARD, cfg.HID), np.float32)
    for c in range(NCORES):
        lo = c * cfg.SHARD
        hi = min(lo + cfg.SHARD, N)
        if hi > lo:
            masks[c, : hi - lo] = mask2[lo:hi]

    meta = dict(
        schedule=schedule, nch=nch, idx_cols=idx_cols, off16=off16,
        calls_b=calls_b, b1_nonzero=b1_nonzero,
    )
    percore = dict(idx=idx128, dn=dn, masks=masks)
    const = dict(iota=iota, ident=ident, W2p=W2p, b1b=b1b)
    return meta, percore, const


# ----------------------------------------------------------------------------
# Program builder (SPMD, one Bass program for all 8 cores)
# ----------------------------------------------------------------------------

def build_program(cfg, meta):
    nc = bacc.Bacc(
        "TRN2", target_bir_lowering=False, debug=False, num_devices=NCORES
    )
    N, T, G, CPB, NBLK = cfg.N, cfg.T, cfg.G, cfg.CPB, cfg.NBLK
    nch = meta["nch"]
    off16 = meta["off16"]
    calls_b = meta["calls_b"]
    schedule = meta["schedule"]
    KH = cfg.HID // P

    x_d = nc.dram_tensor("x_in", [N, cfg.F], F32, kind="ExternalInput")
    w1_d = nc.dram_tensor("w1_in", [cfg.F, cfg.HID], F32, kind="ExternalInput")
    w2p_d = nc.dram_tensor("w2p_in", [P, KH, cfg.OPAD], F32, kind="ExternalInput")
    iota_d = nc.dram_tensor("iota_in", [P, P], F32, kind="ExternalInput")
    ident_d = nc.dram_tensor("ident_in", [P, P], F32, kind="ExternalInput")
    idx_d = nc.dram_tensor("idx_in", [P, meta["idx_cols"]], I16, kind="ExternalInput")
    dn_d = nc.dram_tensor("dn_in", [P, 2 * nch], F32, kind="ExternalInput")
    mask_d = nc.dram_tensor("mask_in", [cfg.SHARD, cfg.HID], F32, kind="ExternalInput")
    if meta["b1_nonzero"]:
        b1b_d = nc.dram_tensor("b1b_in", [P, cfg.HID], F32, kind="ExternalInput")
    hid_d = nc.dram_tensor("hidden_out", [cfg.SHARD, cfg.HID], F32, kind="ExternalOutput")
    out_d = nc.dram_tensor("out2", [cfg.SHARD, cfg.OPAD], F32, kind="ExternalOutput")

    with tile.TileContext(nc) as tc:
        with (
            tc.tile_pool(name="const", bufs=1) as cpool,
            tc.tile_pool(name="dram", bufs=1, space="DRAM") as dram,
        ):
            w1_sb = cpool.tile([P, cfg.HID], F32)
            nc.sync.dma_start(w1_sb[:], w1_d[:, :])
            w2p_sb = cpool.tile([P, KH, cfg.OPAD], F32)
            nc.sync.dma_start(w2p_sb[:], w2p_d[:, :, :])
            iota_sb = cpool.tile([P, P], F32)
            nc.sync.dma_start(iota_sb[:], iota_d[:, :])
            ident_sb = cpool.tile([P, P], F32)
            nc.sync.dma_start(ident_sb[:], ident_d[:, :])
            idx_sb = cpool.tile([P, meta["idx_cols"]], I16)
            nc.sync.dma_start(idx_sb[:], idx_d[:, :])
            dn_sb = cpool.tile([P, 2 * nch], F32)
            nc.sync.dma_start(dn_sb[:], dn_d[:, :])
            if meta["b1_nonzero"]:
                b1b_sb = cpool.tile([P, cfg.HID], F32)
                nc.sync.dma_start(b1b_sb[:], b1b_d[:, :])

            p_bounce = dram.tile([cfg.SHARD, cfg.OPAD], F32)
            p_full = dram.tile([cfg.SHARD * NCORES, cfg.OPAD], F32)

            def s_build(S, gid):
                nc.vector.tensor_scalar(
                    S[:], iota_sb[:],
                    dn_sb[:, 2 * gid : 2 * gid + 1],
                    dn_sb[:, 2 * gid + 1 : 2 * gid + 2],
                    AL.is_equal, AL.mult,
                )

            def run_agg_phase(elem, src_aps, consume_tile):
                """Walk the schedule; gather + aggregate per tile.

                consume_tile(t, psum_get) where psum_get(chunklist, rhs_of_chunk)
                """
                with (
                    tc.tile_pool(name="gath", bufs=3) as gpool,
                    tc.tile_pool(name="spool", bufs=4) as spool,
                ):
                    gtiles = [dict() for _ in range(NBLK)]
                    emitted = [0] * NBLK

                    def ensure(b, call):
                        while emitted[b] <= call:
                            g = emitted[b]
                            buf = gpool.tile([P, CPB, elem], F32, tag=f"g{b}")
                            nc.gpsimd.dma_gather(
                                buf[:],
                                src_aps[b],
                                idx_sb[:, off16[b] + g * (G // 16) : off16[b] + (g + 1) * (G // 16)],
                                G, G, elem,
                                single_packet=(G // 16 + 1 <= 64),
                            )
                            gtiles[b][g] = buf
                            emitted[b] = g + 1

                    for t in range(T):
                        chunks = schedule[t]

                        def rhs_of(b, bchunk):
                            call = bchunk // CPB
                            ensure(b, call)
                            return gtiles[b][call][:, bchunk % CPB, :]

                        consume_tile(t, chunks, rhs_of, spool)

            # ---------------- phase A: layer 1 + hidden + p ----------------
            x_src = [x_d[b * cfg.BLK : min(N, (b + 1) * cfg.BLK), :] for b in range(NBLK)]

            with (
                tc.tile_pool(name="aggT_ps", bufs=2, space="PSUM") as aggT_ps,
                tc.tile_pool(name="h_ps", bufs=2, space="PSUM") as h_ps,
                tc.tile_pool(name="tp_ps", bufs=2, space="PSUM") as tp_ps,
                tc.tile_pool(name="p_ps", bufs=2, space="PSUM") as p_ps,
                tc.tile_pool(name="work", bufs=3) as work,
                tc.tile_pool(name="hidw", bufs=3) as hidw,
            ):
                def tile_a(t, chunks, rhs_of, spool):
                    psum = aggT_ps.tile([P, P], F32)
                    nchk = len(chunks)
                    for ci, (b, gid, bchunk) in enumerate(chunks):
                        S = spool.tile([P, P], F32)
                        s_build(S, gid)
                        nc.tensor.matmul(
                            psum[:], rhs_of(b, bchunk), S[:],
                            start=(ci == 0), stop=(ci == nchk - 1),
                        )
                    aggT_sb = work.tile([P, P], F32, tag="aggT")
                    nc.vector.tensor_copy(aggT_sb[:], psum[:])
                    h_psum = h_ps.tile([P, cfg.HID], F32)
                    nc.tensor.matmul(h_psum[:], aggT_sb[:], w1_sb[:], start=True, stop=True)

                    mask_sb = hidw.tile([P, cfg.HID], F32, tag="mask")
                    nc.sync.dma_start(mask_sb[:], mask_d[t * P : (t + 1) * P, :])
                    hmul = hidw.tile([P, cfg.HID], F32, tag="hmul")
                    if meta["b1_nonzero"]:
                        hb = hidw.tile([P, cfg.HID], F32, tag="hb")
                        nc.vector.tensor_tensor(hb[:], h_psum[:], b1b_sb[:], AL.add)
                        nc.vector.tensor_tensor(hmul[:], hb[:], mask_sb[:], AL.mult)
                    else:
                        nc.vector.tensor_tensor(hmul[:], h_psum[:], mask_sb[:], AL.mult)
                    hid_sb = hidw.tile([P, cfg.HID], F32, tag="hid")
                    nc.scalar.activation(
                        hid_sb[:], hmul[:], mybir.ActivationFunctionType.Relu
                    )
                    nc.sync.dma_start(hid_d[t * P : (t + 1) * P, :], hid_sb[:])

                    p_psum = p_ps.tile([P, cfg.OPAD], F32)
                    for ks in range(KH):
                        tp = tp_ps.tile([P, P], F32)
                        nc.tensor.transpose(tp[:], hid_sb[:, ks * P : (ks + 1) * P], ident_sb[:])
                        hT = work.tile([P, P], F32, tag="hT")
                        nc.vector.tensor_copy(hT[:], tp[:])
                        nc.tensor.matmul(
                            p_psum[:], hT[:], w2p_sb[:, ks, :],
                            start=(ks == 0), stop=(ks == KH - 1),
                        )
                    p_sb = work.tile([P, cfg.OPAD], F32, tag="p")
                    nc.vector.tensor_copy(p_sb[:], p_psum[:])
                    nc.sync.dma_start(p_bounce[t * P : (t + 1) * P, :], p_sb[:])

                run_agg_phase(cfg.F, x_src, tile_a)

            # ---------------- AllGather p ----------------
            nc.gpsimd.collective_compute(
                "AllGather", AL.bypass,
                replica_groups=[list(range(NCORES))],
                ins=[p_bounce.opt()],
                outs=[p_full.opt()],
            )

            # ---------------- phase B: layer 2 aggregation ----------------
            p_src = [
                p_full[b * cfg.BLK : min(cfg.SHARD * NCORES, b * cfg.BLK + cfg.BLK + P), :]
                for b in range(NBLK)
            ]

            with (
                tc.tile_pool(name="o_ps", bufs=2, space="PSUM") as o_ps,
                tc.tile_pool(name="owork", bufs=3) as owork,
            ):
                def tile_b(t, chunks, rhs_of, spool):
                    psum = o_ps.tile([P, cfg.OPAD], F32)
                    nchk = len(chunks)
                    for ci, (b, gid, bchunk) in enumerate(chunks):
                        S = spool.tile([P, P], F32)
                        s_build(S, gid)
                        nc.tensor.matmul(
                            psum[:], S[:], rhs_of(b, bchunk),
                            start=(ci == 0), stop=(ci == nchk - 1),
                        )
                    o_sb = owork.tile([P, cfg.OPAD], F32, tag="o")
                    nc.vector.tensor_copy(o_sb[:], psum[:])
                    nc.sync.dma_start(out_d[t * P : (t + 1) * P, :], o_sb[:])

                run_agg_phase(cfg.OPAD, p_src, tile_b)

    nc.compile()
    return nc


# ----------------------------------------------------------------------------
# Top-level kernel
# ----------------------------------------------------------------------------

def _dropout_mask(cfg):
    """Deterministic dropout mask matching the reference, as {0.,2.} f32."""
    import jax

    cpu = jax.devices("cpu")[0]
    with jax.default_device(cpu):
        keep = jax.random.bernoulli(jax.random.key(42), 0.5, (cfg.N, cfg.HID))
        keep = np.asarray(jax.device_get(keep))
    return keep.astype(np.float32) * 2.0


_CACHE = {}


def _prepare(cfg, edge_index, x, W1, b1, W2, b2):
    key = hash((edge_index.tobytes(), b1.tobytes(), cfg.N, cfg.E))
    hit = _CACHE.get(key)
    if hit is not None:
        return hit
    mask2 = _dropout_mask(cfg)
    meta, percore, const = preprocess(cfg, edge_index, x, W1, b1, W2, b2, mask2)
    nc = build_program(cfg, meta)
    _CACHE[key] = (meta, percore, const, nc)
    return _CACHE[key]


def run(cfg, x, edge_index, W1, b1, W2, b2):
    meta, percore, const, nc = _prepare(cfg, edge_index, x, W1, b1, W2, b2)

    in_maps = []
    for c in range(NCORES):
        m = {
            "x_in": np.ascontiguousarray(x, np.float32),
            "w1_in": np.ascontiguousarray(W1, np.float32),
            "w2p_in": const["W2p"],
            "iota_in": const["iota"],
            "ident_in": const["ident"],
            "idx_in": np.ascontiguousarray(percore["idx"][c]),
            "dn_in": np.ascontiguousarray(percore["dn"][c]),
            "mask_in": np.ascontiguousarray(percore["masks"][c]),
        }
        if meta["b1_nonzero"]:
            m["b1b_in"] = const["b1b"]
        in_maps.append(m)

    old_m = nc.m
    nc.m = get_hw_module(nc.m)
    try:
        res = bass_utils.run_bass_kernel_spmd(
            nc, in_maps, core_ids=list(range(NCORES))
        )
    finally:
        nc.m = old_m

    outs = res.results
    hid_parts, out_parts = [], []
    for c in range(NCORES):
        lo = c * cfg.SHARD
        rows = min(cfg.SHARD, cfg.N - lo)
        hid_parts.append(outs[c]["hidden_out"][:rows])
        out_parts.append(outs[c]["out2"][:rows, : cfg.O])
    hidden = np.concatenate(hid_parts, axis=0)
    out = np.concatenate(out_parts, axis=0) + b2.astype(np.float32)[None, :]
    return out, hidden


def kernel(x, edge_index, W1, b1, W2, b2):
    x = np.asarray(x, np.float32)
    edge_index = np.asarray(edge_index, np.int32)
    W1 = np.asarray(W1, np.float32)
    b1 = np.asarray(b1, np.float32)
    W2 = np.asarray(W2, np.float32)
    b2 = np.asarray(b2, np.float32)
    return run(CFG, x, edge_index, W1, b1, W2, b2)
